# revision 1
# baseline (speedup 1.0000x reference)
"""Bahdanau additive attention on 8 Trainium2 NeuronCores.

Reference computation (per batch b):
  q = query @ W1 + W1_b                  # [t, d]
  k = value @ W2 + W2_b                  # [s, d]
  scores[t,s] = sum_d scale[d] * tanh(q[t,d] + k[s,d])
  scores = where(mask[s], scores, -1e9)
  attn = softmax(scores, axis=s)
  ctx = attn @ value                     # [t, vu]
  returns (ctx, attn)

Sharding: pure data-parallel over batch (b=8 -> 8 cores). Weights replicated.

Per-core kernel layout: d_model on SBUF partitions (4 chunks of 128).
  - projections computed transposed (qT[d,t], kT[d,s]) via PE matmuls
  - q+k broadcast add: tensor_scalar with per-partition scalar, split
    between DVE and GPSIMD (both otherwise idle vs the ACT roofline)
  - tanh: ScalarE ACT on [128, 8*512] tiles (the roofline engine: 1 elem
    per lane per cycle at 1.2 GHz -> ~110us/core minimum)
  - weighted d-reduction: PE matmul, lhsT = zero-padded scale columns
    (bf16), accumulated into a [32, S] PSUM tile over all (t, chunk),
    c-outer so PE pipelines behind ACT tile by tile
  - mask folded in as a host-precomputed additive [64,512] tensor
  - softmax on [t=64 partitions, s=512 free]; exp via ACT with fused bias
    and fused row-sum (accum_out)
  - context: PE transpose of exp(p) (bf16) + 4 bf16 matmuls against value;
    softmax normalization applied after the matmul (ctx = rinv * (p @ v))
  - projection inputs (query/value/W1/W2) are fed in bf16: 4x faster PE
    streaming and half the startup DMA bytes; accumulation stays fp32
  - modeled per-core time (Tile cost model): ~133 us, ACT-roofline-bound
"""

import numpy as np
import ml_dtypes

import concourse.bass as bass
import concourse.tile as tile
from concourse import bacc, mybir
from concourse.bass_utils import run_bass_kernel_spmd

P = 128      # SBUF partitions
T = 64       # query positions per batch
S = 512      # source positions
D = 512      # d_model (= qu = vu)
NCH = 4      # chunks of 128 along d / qu / vu / s
TB = 8       # t-block size for the tanh tiles
TI = 32      # t rows per PSUM score tile (compute APs need 32-aligned bases)
NTI = T // TI
B = 8        # batch == number of cores

F32 = mybir.dt.float32
BF16 = mybir.dt.bfloat16


def build_nc():
    nc = bacc.Bacc(None)

    qT_d = nc.declare_dram_parameter("qT", [P, NCH, T], BF16, isOutput=False)
    vT_d = nc.declare_dram_parameter("vT", [P, NCH, S], BF16, isOutput=False)
    v_d = nc.declare_dram_parameter("v", [P, NCH, D], BF16, isOutput=False)
    w1_d = nc.declare_dram_parameter("W1", [P, NCH, D], BF16, isOutput=False)
    w2_d = nc.declare_dram_parameter("W2", [P, NCH, D], BF16, isOutput=False)
    # scale_pad[p, c, i, j] = scale[c*128+p] * (i == j): a [P, TI, TI] stack of
    # column-padded matrices per chunk. lhsT = scale_pad[:, c, i, :] makes row i
    # of a [TI, S] PSUM tile accumulate t-row i's scores while other rows get +0
    # (accumulating zero is a no-op), keeping the PSUM write base at 0.
    scale_d = nc.declare_dram_parameter("scale_pad", [P, NCH, TI, TI], BF16, isOutput=False)
    b12_d = nc.declare_dram_parameter("b12", [P, NCH], F32, isOutput=False)
    # -(|scale|_1 + 1): a lower bound on -max(scores). softmax is shift
    # invariant, so exp(s - C) with a safe constant C replaces the row-max
    # pass; no under/overflow since |scores| <= |scale|_1.
    negc_d = nc.declare_dram_parameter("negC", [T, 1], F32, isOutput=False)
    mask_d = nc.declare_dram_parameter("maskadd", [T, S], F32, isOutput=False)
    id_d = nc.declare_dram_parameter("ident", [T, T], F32, isOutput=False)
    attn_d = nc.declare_dram_parameter("attn", [T, S], F32, isOutput=True)
    ctx_d = nc.declare_dram_parameter("ctxv", [T, D], F32, isOutput=True)

    Tanh = mybir.ActivationFunctionType.Tanh
    Exp = mybir.ActivationFunctionType.Exp
    X = mybir.AxisListType.X

    with tile.TileContext(nc) as tc:
        with (
            tc.tile_pool(name="persist", bufs=1) as pers,
            tc.tile_pool(name="sump", bufs=4) as sump,
            tc.tile_pool(name="tanhp", bufs=6) as tanhp,
            tc.tile_pool(name="pwork", bufs=5, space=bass.MemorySpace.PSUM) as pwork,
            tc.tile_pool(name="pscore", bufs=2, space=bass.MemorySpace.PSUM) as pscore,
            tc.tile_pool(name="pctx", bufs=1, space=bass.MemorySpace.PSUM) as pctx,
        ):
            w1_sb = pers.tile([P, NCH, D], BF16, tag="w1")
            w2_sb = pers.tile([P, NCH, D], BF16, tag="w2")
            vt_sb = pers.tile([P, NCH, S], BF16, tag="vt")
            v_sb = pers.tile([P, NCH, D], BF16, tag="v")
            qt_sb = pers.tile([P, NCH, T], BF16, tag="qt")
            kt_sb = [
                pers.tile([P, S], F32, tag=f"kt{c}", name=f"kt{c}") for c in range(NCH)
            ]
            qbt_sb = [
                pers.tile([P, T], F32, tag=f"qbt{c}", name=f"qbt{c}")
                for c in range(NCH)
            ]
            scale_sb = pers.tile([P, NCH, TI, TI], BF16, tag="scale")
            b12_sb = pers.tile([P, NCH], F32, tag="b12")
            mask_sb = pers.tile([T, S], F32, tag="mask")
            id_sb = pers.tile([T, T], F32, tag="ident")
            masked_sb = pers.tile([T, S], F32, tag="masked")
            p_sb = pers.tile([T, S], F32, tag="psb")
            attn_sb = pers.tile([T, S], F32, tag="attnw")
            attnT_sb = pers.tile([P, NCH, T], BF16, tag="attnT")
            negc_sb = pers.tile([T, 1], F32, tag="negc")
            rowsum = pers.tile([T, 1], F32, tag="rowsum")
            rinv = pers.tile([T, 1], F32, tag="rinv")
            ctx_sb = pers.tile([T, D], F32, tag="ctxsb")

            # ---- input DMAs ----
            # SP and GPSIMD issue DMAs on separate queues; GPSIMD is idle at
            # startup, so the projection-critical loads split across both
            # (vt on SP, w2 on GPSIMD) and the k-projection starts ~5us in.
            # scale/v/mask arrive later, before their first use.
            nc.sync.dma_start(b12_sb[:], b12_d[:])
            for c in range(NCH):
                nc.sync.dma_start(vt_sb[:, c, :], vT_d[:, c, :])
                nc.gpsimd.dma_start(w2_sb[:, c, :], w2_d[:, c, :])
            for c in range(NCH):
                nc.sync.dma_start(qt_sb[:, c, :], qT_d[:, c, :])
            nc.gpsimd.dma_start(w1_sb[:, 0, :], w1_d[:, 0, :])
            nc.gpsimd.dma_start(w1_sb[:, 1, :], w1_d[:, 1, :])
            nc.sync.dma_start(w1_sb[:, 2, :], w1_d[:, 2, :])
            nc.sync.dma_start(w1_sb[:, 3, :], w1_d[:, 3, :])
            for c in range(NCH):
                nc.sync.dma_start(scale_sb[:, c], scale_d[:, c])
            nc.sync.dma_start(negc_sb[:], negc_d[:])
            nc.sync.dma_start(mask_sb[:], mask_d[:])
            nc.sync.dma_start(v_sb[:], v_d[:])
            nc.sync.dma_start(id_sb[:], id_d[:])

            # ---- projections ----
            # kT[m][d_block, s] = (W2^T v^T)[m];  qbT[m] = (W1^T q^T)[m] + b12.
            # All k-projection matmuls go first (their inputs arrive first);
            # the q-projections fill PE's wait for w1. kt evacuations run on
            # DVE while PE waits for w1; qbt evacuations split DVE/GPSIMD so
            # neither blocks the chunk-0 add pipeline.
            kps, qps = [], []
            for m in range(NCH):
                kp = pwork.tile([P, S], F32, tag="pwork", name="kp")
                for c in range(NCH):
                    nc.tensor.matmul(
                        kp[:],
                        w2_sb[:, c, bass.ts(m, P)],
                        vt_sb[:, c, :],
                        start=(c == 0),
                        stop=(c == NCH - 1),
                    )
                kps.append(kp)
            for m in range(NCH):
                nc.vector.tensor_copy(kt_sb[m][:], kps[m][:])
            for m in range(NCH):
                qp = pwork.tile([P, S], F32, tag="pwork", name="qp")
                for c in range(NCH):
                    nc.tensor.matmul(
                        qp[:, :T],
                        w1_sb[:, c, bass.ts(m, P)],
                        qt_sb[:, c, :],
                        start=(c == 0),
                        stop=(c == NCH - 1),
                    )
                qps.append(qp)

            def qbt_evac(m):
                # PSUM source: DVE only (GPSIMD cannot access PSUM)
                nc.vector.tensor_scalar_add(
                    qbt_sb[m][:], qps[m][:, :T], b12_sb[:, m : m + 1]
                )

            qbt_evac(0)
            qbt_evac(1)

            # ---- main loop: tanh(q+k) and weighted d-reduction ----
            # Per (tbig, sub, c): broadcast-add q column onto kT chunk (DVE and
            # GPSIMD alternate), one big-FD tanh on ACT, then TB matmuls on PE
            # reading only that chunk's tanh tile, so PE trails ACT tile by
            # tile and all of PE's work hides under ACT's.
            NSUB = TI // TB
            cp = pctx.tile([T, D], F32, tag="pctx")
            for tbig in range(NTI):
                ps = pscore.tile([TI, S], F32, tag="pscore")
                for sub in range(NSUB):
                    for c in range(NCH):
                        add_eng = nc.vector if c % 2 == 0 else nc.gpsimd
                        st = sump.tile([P, TB, S], F32, tag="sum")
                        for i in range(TB):
                            t = tbig * TI + sub * TB + i
                            add_eng.tensor_scalar_add(
                                st[:, i, :], kt_sb[c][:], qbt_sb[c][:, t : t + 1]
                            )
                        if tbig == 0 and sub == 0 and c < 2:
                            qbt_evac(c + 2)
                        th = tanhp.tile([P, TB, S], BF16, tag="tanh", name="th")
                        if tbig == 0 and sub == 0 and c == 0:
                            # ladder the very first tanh: small slices start on
                            # ACT after 2 adds instead of all 8
                            nc.scalar.activation(th[:, :2, :], st[:, :2, :], Tanh)
                            nc.scalar.activation(th[:, 2:4, :], st[:, 2:4, :], Tanh)
                            nc.scalar.activation(th[:, 4:, :], st[:, 4:, :], Tanh)
                        elif tbig == NTI - 1 and sub == NSUB - 1 and c == NCH - 1:
                            # ladder the very last tanh: its first 4 matmuls
                            # (and so the softmax tail) start half a tile earlier
                            nc.scalar.activation(th[:, :4, :], st[:, :4, :], Tanh)
                            nc.scalar.activation(th[:, 4:, :], st[:, 4:, :], Tanh)
                        else:
                            nc.scalar.activation(th[:], st[:], Tanh)
                        for i in range(TB):
                            ti = sub * TB + i
                            nc.tensor.matmul(
                                ps[:],
                                scale_sb[:, c, ti, :],
                                th[:, i, :],
                                start=(sub == 0 and c == 0 and i == 0),
                                stop=(sub == NSUB - 1 and c == NCH - 1 and i == TB - 1),
                            )
                # ---- mask + softmax + context for this 32-row half ----
                # (the first half's tail work hides under the second half's
                # main loop; only the last half's is exposed.) The mask-add
                # doubles as the PSUM evacuation; the softmax normalization is
                # applied AFTER the context matmul (ctx = rinv * (p @ v)), so
                # the attn-normalize runs off the critical path.
                h = slice(tbig * TI, (tbig + 1) * TI)
                nc.vector.tensor_add(masked_sb[h, :], ps[:], mask_sb[h, :])
                nc.scalar.activation(
                    p_sb[h, :],
                    masked_sb[h, :],
                    Exp,
                    bias=negc_sb[h, :],
                    accum_out=rowsum[h, :],
                )
                nc.vector.reciprocal(rinv[h, :], rowsum[h, :])
                for j in range(NCH):
                    tp = pwork.tile([P, S], F32, tag="pwork", name="tp")
                    nc.tensor.transpose(tp[:, :TI], p_sb[h, bass.ts(j, P)], id_sb[h, h])
                    # bf16 cast folded into the PSUM evacuation
                    nc.vector.tensor_copy(attnT_sb[:, j, bass.ts(tbig, TI)], tp[:, :TI])
                for j in range(NCH):
                    nc.tensor.matmul(
                        cp[h, :],
                        attnT_sb[:, j, bass.ts(tbig, TI)],
                        v_sb[:, j, :],
                        start=(j == 0),
                        stop=(j == NCH - 1),
                    )
                nc.vector.tensor_scalar_mul(ctx_sb[h, :], cp[h, :], rinv[h, :])
                nc.sync.dma_start(ctx_d[h, :], ctx_sb[h, :])
                nc.vector.tensor_scalar_mul(attn_sb[h, :], p_sb[h, :], rinv[h, :])
                nc.sync.dma_start(attn_d[h, :], attn_sb[h, :])

    nc.compile()
    return nc


def prep_core_inputs(query, value, mask, W1_w, W1_b, W2_w, W2_b, scale):
    """Host-side shard + layout prep. Returns list of 8 per-core input maps."""
    query = np.ascontiguousarray(np.asarray(query, dtype=np.float32))
    value = np.ascontiguousarray(np.asarray(value, dtype=np.float32))
    mask = np.asarray(mask)
    W1_w = np.asarray(W1_w, dtype=np.float32)
    W1_b = np.asarray(W1_b, dtype=np.float32)
    W2_w = np.asarray(W2_w, dtype=np.float32)
    W2_b = np.asarray(W2_b, dtype=np.float32)
    scale = np.asarray(scale, dtype=np.float32)

    # shared across cores; all partition-major [P, NCH, ...] so each SBUF tile
    # loads with a single contiguous DMA
    w1 = np.ascontiguousarray(
        W1_w.reshape(NCH, P, D).transpose(1, 0, 2).astype(ml_dtypes.bfloat16)
    )
    w2 = np.ascontiguousarray(
        W2_w.reshape(NCH, P, D).transpose(1, 0, 2).astype(ml_dtypes.bfloat16)
    )
    scale_pad = np.zeros((P, NCH, TI, TI), dtype=ml_dtypes.bfloat16)
    scale_ch = scale.reshape(NCH, P).astype(ml_dtypes.bfloat16)
    for c in range(NCH):
        for i in range(TI):
            scale_pad[:, c, i, i] = scale_ch[c]
    b12 = np.ascontiguousarray((W1_b + W2_b).reshape(NCH, P).T)
    ident = np.eye(T, dtype=np.float32)
    negc = np.full((T, 1), -(np.abs(scale).sum() + 1.0), dtype=np.float32)

    in_maps = []
    for b in range(B):
        qT = np.ascontiguousarray(
            query[b].T.reshape(NCH, P, T).transpose(1, 0, 2).astype(ml_dtypes.bfloat16)
        )
        vT = np.ascontiguousarray(
            value[b].T.reshape(NCH, P, S).transpose(1, 0, 2).astype(ml_dtypes.bfloat16)
        )
        v = np.ascontiguousarray(
            value[b].reshape(NCH, P, D).transpose(1, 0, 2).astype(ml_dtypes.bfloat16)
        )
        maskadd = np.where(mask[b], np.float32(0.0), np.float32(-1e9))
        maskadd = np.ascontiguousarray(
            np.broadcast_to(maskadd[None, :], (T, S)).astype(np.float32)
        )
        in_maps.append(
            {
                "qT": qT,
                "vT": vT,
                "v": v,
                "W1": w1,
                "W2": w2,
                "scale_pad": scale_pad,
                "b12": b12,
                "negC": negc,
                "maskadd": maskadd,
                "ident": ident,
            }
        )
    return in_maps


_NC_CACHE = None


def _get_nc():
    global _NC_CACHE
    if _NC_CACHE is None:
        _NC_CACHE = build_nc()
    return _NC_CACHE


def run(inputs, trace=False):
    """Run on 8 cores. Returns ((ctx, attn), BassKernelResults)."""
    in_maps = prep_core_inputs(**inputs)
    nc = _get_nc()
    res = run_bass_kernel_spmd(nc, in_maps, list(range(B)), trace=trace)
    ctx = np.stack([res.results[i]["ctxv"] for i in range(B)]).astype(np.float32)
    attn = np.stack([res.results[i]["attn"] for i in range(B)]).astype(np.float32)
    return (ctx, attn), res


def kernel(**inputs):
    (ctx, attn), _ = run(inputs, trace=False)
    return ctx, attn

